# revision 29
# baseline (speedup 1.0000x reference)
"""BlockSparseCausalConv Trainium2 kernel (8 NeuronCores, SPMD).

Sharding: (batch=4) x (time halves=2) across 8 cores. The causal conv needs
only ks-1=15 samples of left history, so time sharding needs no collectives;
per-core outputs are disjoint and the gather is pure concatenation.

Per-core compute: the grouped causal conv for block n is a sum of 16 shifted
64x64 matmuls over its input block-row cols[n]. We:
  - pack 2 taps into one K=128 contraction: SBUF holds each input block-row
    twice (partitions 0:64 raw, 64:128 shifted +1 sample), so a tap offset is
    just a free-dim offset into the same tile; 8 accumulating matmuls per
    (block, 512-time-chunk) in PSUM;
  - pair blocks that share an input block-row into M=128 matmuls (full PE
    array, fast weight load); the pair's two outputs land in PSUM partitions
    0:64 / 64:128;
  - tap-split the leftover unpaired blocks: a single pairs WITH ITSELF --
    M-low takes taps 0-7, M-high takes taps 8-15 of the same moving stream.
    Both halves share the K=128 (2-tap) contraction layout, so a single is
    4 full-array passes per chunk. The high half's products land at output
    t - 8 (later taps from the same stream hit earlier outputs), which the
    drain absorbs as a -8 column offset into the accumulator; the missing
    final 8 columns come from a 4-pass N=8 tail micro-job per single;
  - the scatter-add becomes same-partition-half PSUM->SBUF accumulation on
    the Vector engine into per-(row, half) accumulators; a row written from
    both halves gets two partial buffers that the host sums (cheap).
Passes per 512-time chunk = 8*n_pairs + 4*n_singles = 512 for the graded
input = the dense bf16 PE roofline (524288 cycles/core at 2.4 GHz).

Host side: schedule + weight/x layout prep in numpy, JIT-specialized to the
actual cols/rows values passed in; bf16 matmul operands, fp32 accumulation.
"""
import random
from collections import defaultdict

import numpy as np
import ml_dtypes

import concourse.bacc as bacc
import concourse.mybir as mybir
import concourse.tile as tile
from concourse.bass_utils import run_bass_kernel_spmd

B, C, T = 4, 2048, 2048
NB, BS, KS = 128, 64, 16
NBR = C // BS          # 32 block rows
TH = T // 2            # per-core time span
NT = 512               # matmul moving (time) chunk
NCH = TH // NT         # chunks per core
XW = TH + 16           # per-row x tile width (15 history + 1 shift slack)
N_CORES = 8

# bf16 matmul operands with fp32 PSUM accumulation: rel err ~2.3e-3 vs the
# fp32 reference. (float32r would match PE speed per the cost model but the
# neuronx compile hook in this environment rejects it; plain fp32 is 4x
# slower on the PE.)
_DT = mybir.dt.bfloat16
_NP_DT = ml_dtypes.bfloat16


def _mm(ap):
    return ap

LAST_EXEC_TIME_NS = None


def _build_schedule(cols, rows):
    """Free pairing within each col. Returns:
      pair_jobs: [(nLow, nHigh, col)] full-M jobs, col-grouped emission order
      single_jobs: [s] leftover blocks, each run as a tap-split M=128 job
    Pair orientation chosen greedily to reuse (row, side) accumulator slots
    and balance the two sides (SMAX drives ya's SBUF footprint)."""
    col_blocks = defaultdict(list)
    for n in range(len(cols)):
        col_blocks[int(cols[n])].append(n)

    raw_pairs = []
    single_jobs = []
    for c in sorted(col_blocks):
        blks = sorted(col_blocks[c], key=lambda n: int(rows[n]))
        while len(blks) >= 2:
            raw_pairs.append((blks.pop(), blks.pop(), c))
        if blks:
            single_jobs.append(blks.pop())

    S = [set(), set()]
    for s in single_jobs:  # singles occupy both sides of their row
        S[0].add(int(rows[s]))
        S[1].add(int(rows[s]))
    pair_jobs = []
    for a, b, c in raw_pairs:
        ra, rb = int(rows[a]), int(rows[b])
        cost_ab = (ra not in S[0]) + (rb not in S[1])
        cost_ba = (rb not in S[0]) + (ra not in S[1])
        if cost_ab < cost_ba or (cost_ab == cost_ba and len(S[0]) <= len(S[1])):
            nL, nH = a, b
        else:
            nL, nH = b, a
        S[0].add(int(rows[nL]))
        S[1].add(int(rows[nH]))
        pair_jobs.append((nL, nH, c))
    return pair_jobs, single_jobs


def _emission_order(pair_jobs, single_jobs):
    """Unified job list, singles interleaved ~every 4 pairs so the DVE's
    per-psum drain debt (singles retire a psum every 4 passes, not 8)
    amortizes against the pairs' slack instead of piling up at the end."""
    Jp, Ns = len(pair_jobs), len(single_jobs)
    stride = max(1, Jp // max(Ns, 1))
    emission = []
    si = 0
    for ji, p in enumerate(pair_jobs):
        emission.append(("pair",) + p)
        if (ji + 1) % stride == 0 and si < Ns:
            emission.append(("single", single_jobs[si]))
            si += 1
    while si < Ns:
        emission.append(("single", single_jobs[si]))
        si += 1
    return emission


def _emission_rows(emission, rows):
    """(row, side) occurrences in emission order -> slot maps + first/last
    occurrence index (for copy-vs-accumulate and the output DMA)."""
    occ = []
    for job in emission:
        if job[0] == "pair":
            _, nL, nH, _c = job
            occ.append((int(rows[nL]), 0))
            occ.append((int(rows[nH]), 1))
        else:
            s = job[1]
            occ.append((int(rows[s]), 0))   # taps 0-7 partial
            occ.append((int(rows[s]), 1))   # taps 8-15 (-8 col shift)
    slots = [{}, {}]
    for r, s in occ:
        if r not in slots[s]:
            slots[s][r] = len(slots[s])
    first, last = {}, {}
    for i, k in enumerate(occ):
        if k not in first:
            first[k] = i
        last[k] = i
    return slots, first, last


def _prep_weights(block_values, pair_jobs, single_jobs):
    """lhsT stacks, partition dim first (DMA-friendly):
      wp: (128, Jp, 8, 128)  pair jobs, halves 0:64 / 64:128
      ws: (128, Ns, 4, 128)  tap-split single jobs: pass j has taps
          (2j, 2j+1) in M cols 0:64 and taps (8+2j, 8+2j+1) in 64:128
    lhsT[j][(k2*64+i), oc] = W[n, oc, i, 2*j+k2]."""
    arr = block_values.reshape(NB, BS, BS, 8, 2)             # (n,oc,i,j,k2)
    WT = np.ascontiguousarray(arr.transpose(0, 3, 4, 2, 1))  # (n,j,k2,i,oc)
    WT = WT.reshape(NB, 8, 2 * BS, BS)                       # (n,j,128,64)
    Jp = len(pair_jobs)
    wp = np.zeros((max(Jp, 1), 8, 128, 128), np.float32)
    for ji, (nL, nH, _c) in enumerate(pair_jobs):
        wp[ji, :, :, 0:64] = WT[nL]
        wp[ji, :, :, 64:128] = WT[nH]
    wp = np.ascontiguousarray(wp.transpose(2, 0, 1, 3)).astype(_NP_DT)
    Ns = len(single_jobs)
    ws = np.zeros((max(Ns, 1), 4, 128, 128), np.float32)
    for si, s in enumerate(single_jobs):
        ws[si, :, :, 0:64] = WT[s, 0:4]     # taps 0-7
        ws[si, :, :, 64:128] = WT[s, 4:8]   # taps 8-15
    ws = np.ascontiguousarray(ws.transpose(2, 0, 1, 3)).astype(_NP_DT)
    return wp, ws


def _prep_x_core(x, b, h):
    """(128, NBR, XW) bf16: partitions 0:64 hold x[b, c*64+i, t0-15+u],
    partitions 64:128 the same shifted by +1 sample (zero padded at edges)."""
    t0 = h * TH
    xc = np.zeros((C, XW + 1), np.float32)
    lo, hi = t0 - 15, t0 + TH + 2
    slo, shi = max(lo, 0), min(hi, T)
    xc[:, slo - lo: shi - lo] = x[b, :, slo:shi]
    xr = xc.reshape(NBR, BS, XW + 1)
    xp = np.empty((128, NBR, XW), np.float32)
    xp[0:64] = xr[:, :, 0:XW].transpose(1, 0, 2)
    xp[64:128] = xr[:, :, 1:XW + 1].transpose(1, 0, 2)
    return np.ascontiguousarray(xp).astype(_NP_DT)


def _build_program(pair_jobs, single_jobs, cols, rows, reps=1,
                   no_drain=False):
    """One SPMD Bass program (identical on all 8 cores; data differs).
    reps>1 unrolls the body for repetition-diff timing (first write per
    (row, half) is a copy, so reps are idempotent)."""
    emission = _emission_order(pair_jobs, single_jobs)
    slots, first, last = _emission_rows(emission, rows)
    SMAX = max(len(slots[0]), len(slots[1]), 1)
    Jp, Js = len(pair_jobs), len(single_jobs)

    nc = bacc.Bacc(None, target_bir_lowering=False)
    xd = nc.dram_tensor("xp", [128, NBR, XW], _DT, kind="ExternalInput")
    wdp = nc.dram_tensor("wp", [128, max(Jp, 1), 8, 128], _DT,
                         kind="ExternalInput")
    wds = nc.dram_tensor("ws", [128, max(Js, 1), 4, 128], _DT,
                         kind="ExternalInput")
    yd = nc.dram_tensor("y", [2, SMAX, BS, TH], mybir.dt.float32,
                        kind="ExternalOutput")

    with tile.TileContext(nc) as tc:
        with (
            tc.tile_pool(name="xrows", bufs=6) as xpool,
            tc.tile_pool(name="wts", bufs=6) as wpool,
            tc.tile_pool(name="yacc", bufs=1) as ypool,
            tc.tile_pool(name="psum", bufs=8, space="PSUM") as ppool,
        ):
            ya = ypool.tile([128, SMAX * TH], mybir.dt.float32)

            def drain(ps, occ_idx, r, sidev, ch):
                # per-chunk [64, NT] drains right after each chunk's psum
                # group closes: first write rides the idle ACT engine (it
                # can read PSUM; GPSIMD cannot), accumulates stay on DVE
                if no_drain:
                    return
                s = slots[sidev][r]
                dst = ya[sidev * 64:(sidev + 1) * 64,
                         s * TH + ch * NT: s * TH + ch * NT + NT]
                src = ps[sidev * 64:(sidev + 1) * 64, :]
                if first[(r, sidev)] == occ_idx:
                    nc.scalar.activation(dst, src,
                                         mybir.ActivationFunctionType.Copy)
                else:
                    nc.vector.tensor_add(out=dst, in0=dst, in1=src)
                if last[(r, sidev)] == occ_idx:
                    nc.sync.dma_start(
                        yd[sidev, s, :, ch * NT: ch * NT + NT], dst)

            n_used_cols = len({c for _, _, c in pair_jobs}
                              | {int(cols[s]) for s in single_jobs})
            NXB = n_used_cols + 1   # all cols stay resident within a rep
            NWS = max(Js, 1) + 2    # singles' weights prefetched early
            for _rep in range(reps):
                occ_idx = 0
                xtiles = {}

                def get_x(c, split_first=False):
                    if c in xtiles:
                        return xtiles[c]
                    xt = xpool.tile([128, XW], _DT, tag="xrow", bufs=NXB)
                    if split_first:
                        # split so the first matmuls start sooner
                        half = XW // 2
                        nc.sync.dma_start(xt[:, :half], xd[:, c, :half])
                        nc.sync.dma_start(xt[:, half:], xd[:, c, half:])
                    else:
                        nc.sync.dma_start(xt[:], xd[:, c])
                    xtiles[c] = xt
                    return xt

                # singles' weights are DMA'd one job ahead of use so the
                # interleaved single never waits on the ACT HWDGE queue
                swts = {}
                pi = 0   # pair ordinal (wp index)
                si = 0   # single ordinal (ws index)
                for ei, job in enumerate(emission):
                    if ei + 1 < len(emission) and emission[ei + 1][0] == "single":
                        nsi = sum(1 for jb in emission[:ei + 1]
                                  if jb[0] == "single")
                        swt = wpool.tile([128, 4, 128], _DT, tag="wts2",
                                         bufs=NWS)
                        nc.scalar.dma_start(swt[:], wds[:, nsi])
                        swts[nsi] = swt
                    if job[0] == "pair":
                        _, nL, nH, c = job
                        xrow = get_x(c, split_first=(pi == 0))
                        wt = wpool.tile([128, 8, 128], _DT, tag="wt")
                        if pi == 0:
                            # weights go through the idle ACT engine's HWDGE
                            # queue, in tap-pair chunks, so the first matmul
                            # issues as soon as its lhsT and x halves land
                            for jh in range(4):
                                nc.scalar.dma_start(
                                    wt[:, 2 * jh: 2 * jh + 2],
                                    wdp[:, pi, 2 * jh: 2 * jh + 2])
                        else:
                            nc.scalar.dma_start(wt[:], wdp[:, pi])
                        iL, iH = occ_idx, occ_idx + 1
                        occ_idx += 2
                        for ch in range(NCH):
                            ps = ppool.tile([128, NT], mybir.dt.float32,
                                            tag="ps", bufs=7)
                            for j in range(8):
                                nc.tensor.matmul(
                                    ps[:],
                                    _mm(wt[:, j, :]),
                                    _mm(xrow[:, ch * NT + 2 * j:
                                             ch * NT + 2 * j + NT]),
                                    start=(j == 0),
                                    stop=(j == 7),
                                )
                            drain(ps, iL, int(rows[nL]), 0, ch)
                            drain(ps, iH, int(rows[nH]), 1, ch)
                        pi += 1
                        continue
                    # single: tap-split self-pair, M=128 full array.
                    # M cols 0:64 = taps 0-7 (output-aligned, side 0); cols
                    # 64:128 = taps 8-15, landing at output t-8 (side 1, -8
                    # column shift in the drain; final 8 cols via tail job).
                    s = job[1]
                    c = int(cols[s])
                    r = int(rows[s])
                    x0 = get_x(c)
                    if si in swts:
                        wt = swts.pop(si)
                    else:
                        wt = wpool.tile([128, 4, 128], _DT, tag="wts2",
                                        bufs=NWS)
                        nc.scalar.dma_start(wt[:], wds[:, si])
                    iL, iH = occ_idx, occ_idx + 1
                    occ_idx += 2
                    sl0, sl1 = slots[0][r], slots[1][r]
                    if no_drain:
                        first0 = last0 = first1 = last1 = False
                        emit0 = emit1 = False
                    else:
                        first0, last0 = first[(r, 0)] == iL, last[(r, 0)] == iL
                        first1, last1 = first[(r, 1)] == iH, last[(r, 1)] == iH
                        emit0 = emit1 = True
                    for ch in range(NCH):
                        ps = ppool.tile([128, NT], mybir.dt.float32,
                                        tag="ps", bufs=7)
                        for j in range(4):
                            nc.tensor.matmul(
                                ps[:],
                                _mm(wt[:, j, :]),
                                _mm(x0[:, ch * NT + 2 * j:
                                       ch * NT + 2 * j + NT]),
                                start=(j == 0),
                                stop=(j == 3),
                            )
                        if emit0:
                            dstL = ya[0:64, sl0 * TH + ch * NT:
                                      sl0 * TH + ch * NT + NT]
                            if first0:
                                nc.scalar.activation(
                                    dstL, ps[0:64, :],
                                    mybir.ActivationFunctionType.Copy)
                            else:
                                nc.vector.tensor_add(out=dstL, in0=dstL,
                                                     in1=ps[0:64, :])
                            if last0:
                                nc.sync.dma_start(
                                    yd[0, sl0, :, ch * NT: ch * NT + NT],
                                    dstL)
                        if emit1:
                            # -8 column shift; psum cols mapping to t < t0
                            # belong to the previous core's range: dropped
                            if ch == 0:
                                srcH = ps[64:128, 8:NT]
                                lo, hi = 0, NT - 8
                            else:
                                srcH = ps[64:128, 0:NT]
                                lo, hi = ch * NT - 8, ch * NT + NT - 8
                            dstH = ya[64:128, sl1 * TH + lo: sl1 * TH + hi]
                            if first1:
                                nc.scalar.activation(
                                    dstH, srcH,
                                    mybir.ActivationFunctionType.Copy)
                            else:
                                nc.vector.tensor_add(out=dstH, in0=dstH,
                                                     in1=srcH)
                            if last1:
                                nc.sync.dma_start(yd[1, sl1, :, lo:hi],
                                                  dstH)
                    # tail: taps 8-15 of the final 8 output columns
                    pst = ppool.tile([128, 16], mybir.dt.float32,
                                     tag="pst", bufs=1)
                    for j in range(4):
                        nc.tensor.matmul(
                            pst[64:128, 0:8],
                            _mm(wt[:, j, 64:128]),
                            _mm(x0[:, NCH * NT + 2 * j:
                                   NCH * NT + 2 * j + 8]),
                            start=(j == 0),
                            stop=(j == 3),
                            tile_position=(0, 64),
                            skip_group_check=True,
                        )
                    if emit1:
                        dstT = ya[64:128, sl1 * TH + TH - 8: sl1 * TH + TH]
                        if first1:
                            nc.scalar.activation(
                                dstT, pst[64:128, 0:8],
                                mybir.ActivationFunctionType.Copy)
                        else:
                            nc.vector.tensor_add(out=dstT, in0=dstT,
                                                 in1=pst[64:128, 0:8])
                        if last1:
                            nc.sync.dma_start(yd[1, sl1, :, TH - 8: TH],
                                              dstT)
                    si += 1
    nc.compile()
    return nc, slots


_PROGRAM_CACHE = {}


def kernel(x, block_values, cols, rows):
    global LAST_EXEC_TIME_NS
    x = np.asarray(x)
    block_values = np.asarray(block_values)
    cols = np.asarray(cols)
    rows = np.asarray(rows)
    assert x.shape == (B, C, T) and block_values.shape == (NB, BS, BS, KS)

    pair_jobs, single_jobs = _build_schedule(cols, rows)
    wp, ws = _prep_weights(block_values.astype(np.float32), pair_jobs,
                           single_jobs)
    cache_key = (cols.tobytes(), rows.tobytes())
    if cache_key in _PROGRAM_CACHE:
        nc, slots = _PROGRAM_CACHE[cache_key]
    else:
        nc, slots = _build_program(pair_jobs, single_jobs, cols, rows)
        _PROGRAM_CACHE[cache_key] = (nc, slots)

    in_maps = []
    for core in range(N_CORES):
        b, h = divmod(core, 2)
        in_maps.append({"xp": _prep_x_core(x, b, h), "wp": wp, "ws": ws})

    res = run_bass_kernel_spmd(nc, in_maps, core_ids=list(range(N_CORES)))
    LAST_EXEC_TIME_NS = res.exec_time_ns

    y = np.zeros((B, C, T), np.float32)
    for core in range(N_CORES):
        b, h = divmod(core, 2)
        yc = res.results[core]["y"]  # (2, SMAX, 64, TH)
        for sidev in (0, 1):
            for r, s in slots[sidev].items():
                y[b, r * BS:(r + 1) * BS, h * TH:(h + 1) * TH] += yc[sidev, s]
    return y.astype(x.dtype, copy=False)


if __name__ == "__main__":
    import jax
    import reference

    with jax.default_device(jax.devices("cpu")[0]):
        inputs = reference.setup_inputs()
        np_inputs = {k: np.asarray(v) for k, v in inputs.items()}
        expected = np.asarray(reference.reference(**inputs))
    got = kernel(**np_inputs)
    rel = np.linalg.norm(got - expected) / np.linalg.norm(expected)
    print(f"Relative error: {rel:.3e}")



# revision 30
# speedup vs baseline: 1.1918x; 1.1918x over previous
"""BlockSparseCausalConv Trainium2 kernel (8 NeuronCores, SPMD).

Sharding: (batch=4) x (time halves=2) across 8 cores. The causal conv needs
only ks-1=15 samples of left history, so time sharding needs no collectives;
per-core outputs are disjoint and the gather is pure concatenation.

Per-core compute: the grouped causal conv for block n is a sum of 16 shifted
64x64 matmuls over its input block-row cols[n]. We:
  - pack 2 taps into one K=128 contraction: SBUF holds each input block-row
    twice (partitions 0:64 raw, 64:128 shifted +1 sample), so a tap offset is
    just a free-dim offset into the same tile; 8 accumulating matmuls per
    (block, 512-time-chunk) in PSUM;
  - pair blocks that share an input block-row into M=128 matmuls (full PE
    array, fast weight load); the pair's two outputs land in PSUM partitions
    0:64 / 64:128;
  - tap-split the leftover unpaired blocks: a single pairs WITH ITSELF --
    M-low takes taps 0-7, M-high takes taps 8-15 of the same moving stream.
    Both halves share the K=128 (2-tap) contraction layout, so a single is
    4 full-array passes per chunk. The high half's products land at output
    t - 8 (later taps from the same stream hit earlier outputs), which the
    drain absorbs as a -8 column offset into the accumulator; the missing
    final 8 columns come from a 4-pass N=8 tail micro-job per single;
  - the scatter-add becomes same-partition-half PSUM->SBUF accumulation
    into per-(row, half) accumulators: first writes ride the otherwise-idle
    ACT engine (Copy activation, PSUM-capable), accumulates run on the DVE;
    a row written from both halves gets two partial buffers the host sums.
  - singles are interleaved ~every 4 pairs so the DVE's higher per-psum
    drain rate on singles amortizes against the pairs' slack; all x block
    rows stay SBUF-resident for the whole rep, and singles' weights are
    DMA'd one job ahead, so no job ever waits on DMA.
Passes per 512-time chunk = 8*n_pairs + 4*n_singles = 512 for the graded
input = the dense bf16 PE roofline (524288 cycles/core at 2.4 GHz).

Host side: schedule + weight/x layout prep in numpy, JIT-specialized to the
actual cols/rows values passed in; bf16 matmul operands, fp32 accumulation.
"""
from collections import defaultdict

import numpy as np
import ml_dtypes

import concourse.bacc as bacc
import concourse.mybir as mybir
import concourse.tile as tile
from concourse.bass_utils import run_bass_kernel_spmd

B, C, T = 4, 2048, 2048
NB, BS, KS = 128, 64, 16
NBR = C // BS          # 32 block rows
TH = T // 2            # per-core time span
NT = 512               # matmul moving (time) chunk
NCH = TH // NT         # chunks per core
XW = TH + 16           # per-row x tile width (15 history + 1 shift slack)
N_CORES = 8

# bf16 matmul operands with fp32 PSUM accumulation: rel err ~2.3e-3 vs the
# fp32 reference. (float32r would match PE speed per the cost model but the
# neuronx compile hook in this environment rejects it; plain fp32 is 4x
# slower on the PE.)
_DT = mybir.dt.bfloat16
_NP_DT = ml_dtypes.bfloat16


def _mm(ap):
    return ap

LAST_EXEC_TIME_NS = None


def _build_schedule(cols, rows):
    """Free pairing within each col. Returns:
      pair_jobs: [(nLow, nHigh, col)] full-M jobs, col-grouped emission order
      single_jobs: [s] leftover blocks, each run as a tap-split M=128 job
    Pair orientation chosen greedily to reuse (row, side) accumulator slots
    and balance the two sides (SMAX drives ya's SBUF footprint)."""
    col_blocks = defaultdict(list)
    for n in range(len(cols)):
        col_blocks[int(cols[n])].append(n)

    raw_pairs = []
    single_jobs = []
    for c in sorted(col_blocks):
        blks = sorted(col_blocks[c], key=lambda n: int(rows[n]))
        while len(blks) >= 2:
            raw_pairs.append((blks.pop(), blks.pop(), c))
        if blks:
            single_jobs.append(blks.pop())

    S = [set(), set()]
    for s in single_jobs:  # singles occupy both sides of their row
        S[0].add(int(rows[s]))
        S[1].add(int(rows[s]))
    pair_jobs = []
    for a, b, c in raw_pairs:
        ra, rb = int(rows[a]), int(rows[b])
        cost_ab = (ra not in S[0]) + (rb not in S[1])
        cost_ba = (rb not in S[0]) + (ra not in S[1])
        if cost_ab < cost_ba or (cost_ab == cost_ba and len(S[0]) <= len(S[1])):
            nL, nH = a, b
        else:
            nL, nH = b, a
        S[0].add(int(rows[nL]))
        S[1].add(int(rows[nH]))
        pair_jobs.append((nL, nH, c))
    return pair_jobs, single_jobs


def _emission_order(pair_jobs, single_jobs):
    """Unified job list, singles interleaved ~every 4 pairs so the DVE's
    per-psum drain debt (singles retire a psum every 4 passes, not 8)
    amortizes against the pairs' slack instead of piling up at the end."""
    Jp, Ns = len(pair_jobs), len(single_jobs)
    stride = max(1, Jp // max(Ns, 1))
    emission = []
    si = 0
    for ji, p in enumerate(pair_jobs):
        emission.append(("pair",) + p)
        if (ji + 1) % stride == 0 and si < Ns:
            emission.append(("single", single_jobs[si]))
            si += 1
    while si < Ns:
        emission.append(("single", single_jobs[si]))
        si += 1
    return emission


def _emission_rows(emission, rows):
    """(row, side) occurrences in emission order -> slot maps + first/last
    occurrence index (for copy-vs-accumulate and the output DMA)."""
    occ = []
    for job in emission:
        if job[0] == "pair":
            _, nL, nH, _c = job
            occ.append((int(rows[nL]), 0))
            occ.append((int(rows[nH]), 1))
        else:
            s = job[1]
            occ.append((int(rows[s]), 0))   # taps 0-7 partial
            occ.append((int(rows[s]), 1))   # taps 8-15 (-8 col shift)
    slots = [{}, {}]
    for r, s in occ:
        if r not in slots[s]:
            slots[s][r] = len(slots[s])
    first, last = {}, {}
    for i, k in enumerate(occ):
        if k not in first:
            first[k] = i
        last[k] = i
    return slots, first, last


def _prep_weights(block_values, pair_jobs, single_jobs):
    """lhsT stacks, partition dim first (DMA-friendly):
      wp: (128, Jp, 8, 128)  pair jobs, halves 0:64 / 64:128
      ws: (128, Ns, 4, 128)  tap-split single jobs: pass j has taps
          (2j, 2j+1) in M cols 0:64 and taps (8+2j, 8+2j+1) in 64:128
    lhsT[j][(k2*64+i), oc] = W[n, oc, i, 2*j+k2]."""
    arr = block_values.reshape(NB, BS, BS, 8, 2)             # (n,oc,i,j,k2)
    WT = np.ascontiguousarray(arr.transpose(0, 3, 4, 2, 1))  # (n,j,k2,i,oc)
    WT = WT.reshape(NB, 8, 2 * BS, BS)                       # (n,j,128,64)
    Jp = len(pair_jobs)
    wp = np.zeros((max(Jp, 1), 8, 128, 128), np.float32)
    for ji, (nL, nH, _c) in enumerate(pair_jobs):
        wp[ji, :, :, 0:64] = WT[nL]
        wp[ji, :, :, 64:128] = WT[nH]
    wp = np.ascontiguousarray(wp.transpose(2, 0, 1, 3)).astype(_NP_DT)
    Ns = len(single_jobs)
    ws = np.zeros((max(Ns, 1), 4, 128, 128), np.float32)
    for si, s in enumerate(single_jobs):
        ws[si, :, :, 0:64] = WT[s, 0:4]     # taps 0-7
        ws[si, :, :, 64:128] = WT[s, 4:8]   # taps 8-15
    ws = np.ascontiguousarray(ws.transpose(2, 0, 1, 3)).astype(_NP_DT)
    return wp, ws


def _prep_x_core(x, b, h):
    """(128, NBR, XW) bf16: partitions 0:64 hold x[b, c*64+i, t0-15+u],
    partitions 64:128 the same shifted by +1 sample (zero padded at edges)."""
    t0 = h * TH
    xc = np.zeros((C, XW + 1), np.float32)
    lo, hi = t0 - 15, t0 + TH + 2
    slo, shi = max(lo, 0), min(hi, T)
    xc[:, slo - lo: shi - lo] = x[b, :, slo:shi]
    xr = xc.reshape(NBR, BS, XW + 1)
    xp = np.empty((128, NBR, XW), np.float32)
    xp[0:64] = xr[:, :, 0:XW].transpose(1, 0, 2)
    xp[64:128] = xr[:, :, 1:XW + 1].transpose(1, 0, 2)
    return np.ascontiguousarray(xp).astype(_NP_DT)


def _build_program(pair_jobs, single_jobs, cols, rows, reps=1,
                   no_drain=False):
    """One SPMD Bass program (identical on all 8 cores; data differs).
    reps>1 unrolls the body for repetition-diff timing (first write per
    (row, half) is a copy, so reps are idempotent)."""
    emission = _emission_order(pair_jobs, single_jobs)
    slots, first, last = _emission_rows(emission, rows)
    SMAX = max(len(slots[0]), len(slots[1]), 1)
    Jp, Js = len(pair_jobs), len(single_jobs)

    nc = bacc.Bacc(None, target_bir_lowering=False)
    xd = nc.dram_tensor("xp", [128, NBR, XW], _DT, kind="ExternalInput")
    wdp = nc.dram_tensor("wp", [128, max(Jp, 1), 8, 128], _DT,
                         kind="ExternalInput")
    wds = nc.dram_tensor("ws", [128, max(Js, 1), 4, 128], _DT,
                         kind="ExternalInput")
    yd = nc.dram_tensor("y", [2, SMAX, BS, TH], mybir.dt.float32,
                        kind="ExternalOutput")

    with tile.TileContext(nc) as tc:
        with (
            tc.tile_pool(name="xrows", bufs=6) as xpool,
            tc.tile_pool(name="wts", bufs=6) as wpool,
            tc.tile_pool(name="yacc", bufs=1) as ypool,
            tc.tile_pool(name="psum", bufs=8, space="PSUM") as ppool,
        ):
            ya = ypool.tile([128, SMAX * TH], mybir.dt.float32)

            def drain(ps, occ_idx, r, sidev, ch):
                # per-chunk [64, NT] drains right after each chunk's psum
                # group closes: first write rides the idle ACT engine (it
                # can read PSUM; GPSIMD cannot), accumulates stay on DVE
                if no_drain:
                    return
                s = slots[sidev][r]
                dst = ya[sidev * 64:(sidev + 1) * 64,
                         s * TH + ch * NT: s * TH + ch * NT + NT]
                src = ps[sidev * 64:(sidev + 1) * 64, :]
                if first[(r, sidev)] == occ_idx:
                    nc.scalar.activation(dst, src,
                                         mybir.ActivationFunctionType.Copy)
                else:
                    nc.vector.tensor_add(out=dst, in0=dst, in1=src)
                if last[(r, sidev)] == occ_idx:
                    nc.sync.dma_start(
                        yd[sidev, s, :, ch * NT: ch * NT + NT], dst)

            n_used_cols = len({c for _, _, c in pair_jobs}
                              | {int(cols[s]) for s in single_jobs})
            NXB = n_used_cols + 1   # all cols stay resident within a rep
            NWS = max(Js, 1) + 2    # singles' weights prefetched early
            for _rep in range(reps):
                occ_idx = 0
                xtiles = {}

                def get_x(c, split_first=False):
                    if c in xtiles:
                        return xtiles[c]
                    xt = xpool.tile([128, XW], _DT, tag="xrow", bufs=NXB)
                    if split_first:
                        # split so the first matmuls start sooner
                        half = XW // 2
                        nc.sync.dma_start(xt[:, :half], xd[:, c, :half])
                        nc.sync.dma_start(xt[:, half:], xd[:, c, half:])
                    else:
                        nc.sync.dma_start(xt[:], xd[:, c])
                    xtiles[c] = xt
                    return xt

                # singles' weights are DMA'd one job ahead of use so the
                # interleaved single never waits on the ACT HWDGE queue
                swts = {}
                pi = 0   # pair ordinal (wp index)
                si = 0   # single ordinal (ws index)
                for ei, job in enumerate(emission):
                    if ei + 1 < len(emission) and emission[ei + 1][0] == "single":
                        nsi = sum(1 for jb in emission[:ei + 1]
                                  if jb[0] == "single")
                        swt = wpool.tile([128, 4, 128], _DT, tag="wts2",
                                         bufs=NWS)
                        nc.scalar.dma_start(swt[:], wds[:, nsi])
                        swts[nsi] = swt
                    if job[0] == "pair":
                        _, nL, nH, c = job
                        xrow = get_x(c, split_first=(pi == 0))
                        wt = wpool.tile([128, 8, 128], _DT, tag="wt")
                        if pi == 0:
                            # weights go through the idle ACT engine's HWDGE
                            # queue, in tap-pair chunks, so the first matmul
                            # issues as soon as its lhsT and x halves land
                            for jh in range(4):
                                nc.scalar.dma_start(
                                    wt[:, 2 * jh: 2 * jh + 2],
                                    wdp[:, pi, 2 * jh: 2 * jh + 2])
                        else:
                            nc.scalar.dma_start(wt[:], wdp[:, pi])
                        iL, iH = occ_idx, occ_idx + 1
                        occ_idx += 2
                        for ch in range(NCH):
                            ps = ppool.tile([128, NT], mybir.dt.float32,
                                            tag="ps", bufs=7)
                            for j in range(8):
                                nc.tensor.matmul(
                                    ps[:],
                                    _mm(wt[:, j, :]),
                                    _mm(xrow[:, ch * NT + 2 * j:
                                             ch * NT + 2 * j + NT]),
                                    start=(j == 0),
                                    stop=(j == 7),
                                )
                            drain(ps, iL, int(rows[nL]), 0, ch)
                            drain(ps, iH, int(rows[nH]), 1, ch)
                        pi += 1
                        continue
                    # single: tap-split self-pair, M=128 full array.
                    # M cols 0:64 = taps 0-7 (output-aligned, side 0); cols
                    # 64:128 = taps 8-15, landing at output t-8 (side 1, -8
                    # column shift in the drain; final 8 cols via tail job).
                    s = job[1]
                    c = int(cols[s])
                    r = int(rows[s])
                    x0 = get_x(c)
                    if si in swts:
                        wt = swts.pop(si)
                    else:
                        wt = wpool.tile([128, 4, 128], _DT, tag="wts2",
                                        bufs=NWS)
                        nc.scalar.dma_start(wt[:], wds[:, si])
                    iL, iH = occ_idx, occ_idx + 1
                    occ_idx += 2
                    sl0, sl1 = slots[0][r], slots[1][r]
                    if no_drain:
                        first0 = last0 = first1 = last1 = False
                        emit0 = emit1 = False
                    else:
                        first0, last0 = first[(r, 0)] == iL, last[(r, 0)] == iL
                        first1, last1 = first[(r, 1)] == iH, last[(r, 1)] == iH
                        emit0 = emit1 = True
                    for ch in range(NCH):
                        ps = ppool.tile([128, NT], mybir.dt.float32,
                                        tag="ps", bufs=7)
                        for j in range(4):
                            nc.tensor.matmul(
                                ps[:],
                                _mm(wt[:, j, :]),
                                _mm(x0[:, ch * NT + 2 * j:
                                       ch * NT + 2 * j + NT]),
                                start=(j == 0),
                                stop=(j == 3),
                            )
                        if emit0:
                            dstL = ya[0:64, sl0 * TH + ch * NT:
                                      sl0 * TH + ch * NT + NT]
                            if first0:
                                nc.scalar.activation(
                                    dstL, ps[0:64, :],
                                    mybir.ActivationFunctionType.Copy)
                            else:
                                nc.vector.tensor_add(out=dstL, in0=dstL,
                                                     in1=ps[0:64, :])
                            if last0:
                                nc.sync.dma_start(
                                    yd[0, sl0, :, ch * NT: ch * NT + NT],
                                    dstL)
                        if emit1:
                            # -8 column shift; psum cols mapping to t < t0
                            # belong to the previous core's range: dropped
                            if ch == 0:
                                srcH = ps[64:128, 8:NT]
                                lo, hi = 0, NT - 8
                            else:
                                srcH = ps[64:128, 0:NT]
                                lo, hi = ch * NT - 8, ch * NT + NT - 8
                            dstH = ya[64:128, sl1 * TH + lo: sl1 * TH + hi]
                            if first1:
                                nc.scalar.activation(
                                    dstH, srcH,
                                    mybir.ActivationFunctionType.Copy)
                            else:
                                nc.vector.tensor_add(out=dstH, in0=dstH,
                                                     in1=srcH)
                            if last1:
                                nc.sync.dma_start(yd[1, sl1, :, lo:hi],
                                                  dstH)
                    # tail: taps 8-15 of the final 8 output columns
                    pst = ppool.tile([128, 16], mybir.dt.float32,
                                     tag="pst", bufs=1)
                    for j in range(4):
                        nc.tensor.matmul(
                            pst[64:128, 0:8],
                            _mm(wt[:, j, 64:128]),
                            _mm(x0[:, NCH * NT + 2 * j:
                                   NCH * NT + 2 * j + 8]),
                            start=(j == 0),
                            stop=(j == 3),
                            tile_position=(0, 64),
                            skip_group_check=True,
                        )
                    if emit1:
                        dstT = ya[64:128, sl1 * TH + TH - 8: sl1 * TH + TH]
                        if first1:
                            nc.scalar.activation(
                                dstT, pst[64:128, 0:8],
                                mybir.ActivationFunctionType.Copy)
                        else:
                            nc.vector.tensor_add(out=dstT, in0=dstT,
                                                 in1=pst[64:128, 0:8])
                        if last1:
                            nc.sync.dma_start(yd[1, sl1, :, TH - 8: TH],
                                              dstT)
                    si += 1
    nc.compile()
    return nc, slots


_PROGRAM_CACHE = {}


def kernel(x, block_values, cols, rows):
    global LAST_EXEC_TIME_NS
    x = np.asarray(x)
    block_values = np.asarray(block_values)
    cols = np.asarray(cols)
    rows = np.asarray(rows)
    assert x.shape == (B, C, T) and block_values.shape == (NB, BS, BS, KS)

    pair_jobs, single_jobs = _build_schedule(cols, rows)
    wp, ws = _prep_weights(block_values.astype(np.float32), pair_jobs,
                           single_jobs)
    cache_key = (cols.tobytes(), rows.tobytes())
    if cache_key in _PROGRAM_CACHE:
        nc, slots = _PROGRAM_CACHE[cache_key]
    else:
        nc, slots = _build_program(pair_jobs, single_jobs, cols, rows)
        _PROGRAM_CACHE[cache_key] = (nc, slots)

    in_maps = []
    for core in range(N_CORES):
        b, h = divmod(core, 2)
        in_maps.append({"xp": _prep_x_core(x, b, h), "wp": wp, "ws": ws})

    res = run_bass_kernel_spmd(nc, in_maps, core_ids=list(range(N_CORES)))
    LAST_EXEC_TIME_NS = res.exec_time_ns

    y = np.zeros((B, C, T), np.float32)
    for core in range(N_CORES):
        b, h = divmod(core, 2)
        yc = res.results[core]["y"]  # (2, SMAX, 64, TH)
        for sidev in (0, 1):
            for r, s in slots[sidev].items():
                y[b, r * BS:(r + 1) * BS, h * TH:(h + 1) * TH] += yc[sidev, s]
    return y.astype(x.dtype, copy=False)


if __name__ == "__main__":
    import jax
    import reference

    with jax.default_device(jax.devices("cpu")[0]):
        inputs = reference.setup_inputs()
        np_inputs = {k: np.asarray(v) for k, v in inputs.items()}
        expected = np.asarray(reference.reference(**inputs))
    got = kernel(**np_inputs)
    rel = np.linalg.norm(got - expected) / np.linalg.norm(expected)
    print(f"Relative error: {rel:.3e}")



# revision 32
# speedup vs baseline: 1.2961x; 1.0876x over previous
"""BlockSparseCausalConv Trainium2 kernel (8 NeuronCores, SPMD).

Sharding: (batch=4) x (time halves=2) across 8 cores. The causal conv needs
only ks-1=15 samples of left history, so time sharding needs no collectives;
per-core outputs are disjoint and the gather is pure concatenation.

Per-core compute: the grouped causal conv for block n is a sum of 16 shifted
64x64 matmuls over its input block-row cols[n]. We:
  - pack 2 taps into one K=128 contraction: SBUF holds each input block-row
    twice (partitions 0:64 raw, 64:128 shifted +1 sample), so a tap offset is
    just a free-dim offset into the same tile; 8 accumulating matmuls per
    (block, 512-time-chunk) in PSUM;
  - pair blocks that share an input block-row into M=128 matmuls (full PE
    array, fast weight load); the pair's two outputs land in PSUM partitions
    0:64 / 64:128;
  - tap-split the leftover unpaired blocks: a single pairs WITH ITSELF --
    M-low takes taps 0-7, M-high takes taps 8-15 of the same moving stream.
    Both halves share the K=128 (2-tap) contraction layout, so a single is
    4 full-array passes per chunk. The high half's products land at output
    t - 8 (later taps from the same stream hit earlier outputs), which the
    drain absorbs as a -8 column offset into the accumulator; the missing
    final 8 columns come from a 4-pass N=8 tail micro-job per single;
  - the scatter-add becomes same-partition-half PSUM->SBUF accumulation
    into per-(row, half) accumulators: first writes ride the otherwise-idle
    ACT engine (Copy activation, PSUM-capable), accumulates run on the DVE;
    a row written from both halves gets two partial buffers the host sums.
  - singles are interleaved ~every 4 pairs so the DVE's higher per-psum
    drain rate on singles amortizes against the pairs' slack; all x block
    rows stay SBUF-resident for the whole rep, and singles' weights are
    DMA'd one job ahead, so no job ever waits on DMA.
Passes per 512-time chunk = 8*n_pairs + 4*n_singles = 512 for the graded
input = the dense bf16 PE roofline (524288 cycles/core at 2.4 GHz).

Host side: schedule + weight/x layout prep in numpy, JIT-specialized to the
actual cols/rows values passed in; bf16 matmul operands, fp32 accumulation.
"""
from collections import defaultdict

import numpy as np
import ml_dtypes

import concourse.bacc as bacc
import concourse.mybir as mybir
import concourse.tile as tile
from concourse.bass_utils import run_bass_kernel_spmd

B, C, T = 4, 2048, 2048
NB, BS, KS = 128, 64, 16
NBR = C // BS          # 32 block rows
TH = T // 2            # per-core time span
NT = 512               # matmul moving (time) chunk
NCH = TH // NT         # chunks per core
XW = TH + 16           # per-row x tile width (15 history + 1 shift slack)
N_CORES = 8

# bf16 matmul operands with fp32 PSUM accumulation: rel err ~2.3e-3 vs the
# fp32 reference. (float32r would match PE speed per the cost model but the
# neuronx compile hook in this environment rejects it; plain fp32 is 4x
# slower on the PE.)
_DT = mybir.dt.bfloat16
_NP_DT = ml_dtypes.bfloat16


def _mm(ap):
    return ap

LAST_EXEC_TIME_NS = None


def _build_schedule(cols, rows):
    """Free pairing within each col. Returns:
      pair_jobs: [(nLow, nHigh, col)] full-M jobs, col-grouped emission order
      single_jobs: [s] leftover blocks, each run as a tap-split M=128 job
    Pair orientation chosen greedily to reuse (row, side) accumulator slots
    and balance the two sides (SMAX drives ya's SBUF footprint)."""
    col_blocks = defaultdict(list)
    for n in range(len(cols)):
        col_blocks[int(cols[n])].append(n)

    raw_pairs = []
    single_jobs = []
    for c in sorted(col_blocks):
        blks = sorted(col_blocks[c], key=lambda n: int(rows[n]))
        while len(blks) >= 2:
            raw_pairs.append((blks.pop(), blks.pop(), c))
        if blks:
            single_jobs.append(blks.pop())

    S = [set(), set()]
    for s in single_jobs:  # singles occupy both sides of their row
        S[0].add(int(rows[s]))
        S[1].add(int(rows[s]))
    pair_jobs = []
    for a, b, c in raw_pairs:
        ra, rb = int(rows[a]), int(rows[b])
        cost_ab = (ra not in S[0]) + (rb not in S[1])
        cost_ba = (rb not in S[0]) + (ra not in S[1])
        if cost_ab < cost_ba or (cost_ab == cost_ba and len(S[0]) <= len(S[1])):
            nL, nH = a, b
        else:
            nL, nH = b, a
        S[0].add(int(rows[nL]))
        S[1].add(int(rows[nH]))
        pair_jobs.append((nL, nH, c))
    return pair_jobs, single_jobs


def _emission_order(pair_jobs, single_jobs):
    """Unified job list, singles interleaved ~every 4 pairs so the DVE's
    per-psum drain debt (singles retire a psum every 4 passes, not 8)
    amortizes against the pairs' slack instead of piling up at the end."""
    Jp, Ns = len(pair_jobs), len(single_jobs)
    stride = max(1, Jp // max(Ns, 1))
    emission = []
    si = 0
    for ji, p in enumerate(pair_jobs):
        emission.append(("pair",) + p)
        if (ji + 1) % stride == 0 and si < Ns:
            emission.append(("single", single_jobs[si]))
            si += 1
    while si < Ns:
        emission.append(("single", single_jobs[si]))
        si += 1
    return emission


def _emission_rows(emission, rows):
    """(row, side) occurrences in emission order -> slot maps + first/last
    occurrence index (for copy-vs-accumulate and the output DMA)."""
    occ = []
    for job in emission:
        if job[0] == "pair":
            _, nL, nH, _c = job
            occ.append((int(rows[nL]), 0))
            occ.append((int(rows[nH]), 1))
        else:
            s = job[1]
            occ.append((int(rows[s]), 0))   # taps 0-7 partial
            occ.append((int(rows[s]), 1))   # taps 8-15 (-8 col shift)
    slots = [{}, {}]
    for r, s in occ:
        if r not in slots[s]:
            slots[s][r] = len(slots[s])
    first, last = {}, {}
    for i, k in enumerate(occ):
        if k not in first:
            first[k] = i
        last[k] = i
    return slots, first, last


def _prep_weights(block_values, pair_jobs, single_jobs):
    """lhsT stacks, partition dim first (DMA-friendly):
      wp: (128, Jp, 8, 128)  pair jobs, halves 0:64 / 64:128
      ws: (128, Ns, 4, 128)  tap-split single jobs: pass j has taps
          (2j, 2j+1) in M cols 0:64 and taps (8+2j, 8+2j+1) in 64:128
    lhsT[j][(k2*64+i), oc] = W[n, oc, i, 2*j+k2]."""
    arr = block_values.reshape(NB, BS, BS, 8, 2)             # (n,oc,i,j,k2)
    WT = np.ascontiguousarray(arr.transpose(0, 3, 4, 2, 1))  # (n,j,k2,i,oc)
    WT = WT.reshape(NB, 8, 2 * BS, BS)                       # (n,j,128,64)
    Jp = len(pair_jobs)
    wp = np.zeros((max(Jp, 1), 8, 128, 128), np.float32)
    for ji, (nL, nH, _c) in enumerate(pair_jobs):
        wp[ji, :, :, 0:64] = WT[nL]
        wp[ji, :, :, 64:128] = WT[nH]
    wp = np.ascontiguousarray(wp.transpose(2, 0, 1, 3)).astype(_NP_DT)
    Ns = len(single_jobs)
    ws = np.zeros((max(Ns, 1), 4, 128, 128), np.float32)
    for si, s in enumerate(single_jobs):
        ws[si, :, :, 0:64] = WT[s, 0:4]     # taps 0-7
        ws[si, :, :, 64:128] = WT[s, 4:8]   # taps 8-15
    ws = np.ascontiguousarray(ws.transpose(2, 0, 1, 3)).astype(_NP_DT)
    return wp, ws


def _prep_x_core(x, b, h):
    """(128, NBR, XW) bf16: partitions 0:64 hold x[b, c*64+i, t0-15+u],
    partitions 64:128 the same shifted by +1 sample (zero padded at edges)."""
    t0 = h * TH
    xc = np.zeros((C, XW + 1), np.float32)
    lo, hi = t0 - 15, t0 + TH + 2
    slo, shi = max(lo, 0), min(hi, T)
    xc[:, slo - lo: shi - lo] = x[b, :, slo:shi]
    xr = xc.reshape(NBR, BS, XW + 1)
    xp = np.empty((128, NBR, XW), np.float32)
    xp[0:64] = xr[:, :, 0:XW].transpose(1, 0, 2)
    xp[64:128] = xr[:, :, 1:XW + 1].transpose(1, 0, 2)
    return np.ascontiguousarray(xp).astype(_NP_DT)


def _build_program(pair_jobs, single_jobs, cols, rows, reps=1,
                   no_drain=False):
    """One SPMD Bass program (identical on all 8 cores; data differs).
    reps>1 unrolls the body for repetition-diff timing (first write per
    (row, half) is a copy, so reps are idempotent)."""
    emission = _emission_order(pair_jobs, single_jobs)
    slots, first, last = _emission_rows(emission, rows)
    SMAX = max(len(slots[0]), len(slots[1]), 1)
    Jp, Js = len(pair_jobs), len(single_jobs)

    nc = bacc.Bacc(None, target_bir_lowering=False)
    xd = nc.dram_tensor("xp", [128, NBR, XW], _DT, kind="ExternalInput")
    wdp = nc.dram_tensor("wp", [128, max(Jp, 1), 8, 128], _DT,
                         kind="ExternalInput")
    wds = nc.dram_tensor("ws", [128, max(Js, 1), 4, 128], _DT,
                         kind="ExternalInput")
    yd = nc.dram_tensor("y", [2, SMAX, BS, TH], mybir.dt.float32,
                        kind="ExternalOutput")

    with tile.TileContext(nc) as tc:
        with (
            tc.tile_pool(name="xrows", bufs=6) as xpool,
            tc.tile_pool(name="wts", bufs=6) as wpool,
            tc.tile_pool(name="yacc", bufs=1) as ypool,
            tc.tile_pool(name="psum", bufs=8, space="PSUM") as ppool,
        ):
            ya = ypool.tile([128, SMAX * TH], mybir.dt.float32)

            def drain(ps, occ_idx, r, sidev, ch):
                # per-chunk [64, NT] drains right after each chunk's psum
                # group closes: first write rides the idle ACT engine (it
                # can read PSUM; GPSIMD cannot), accumulates stay on DVE
                if no_drain:
                    return
                s = slots[sidev][r]
                dst = ya[sidev * 64:(sidev + 1) * 64,
                         s * TH + ch * NT: s * TH + ch * NT + NT]
                src = ps[sidev * 64:(sidev + 1) * 64, :]
                if first[(r, sidev)] == occ_idx:
                    nc.scalar.activation(dst, src,
                                         mybir.ActivationFunctionType.Copy)
                else:
                    nc.vector.tensor_add(out=dst, in0=dst, in1=src)
                if last[(r, sidev)] == occ_idx:
                    nc.sync.dma_start(
                        yd[sidev, s, :, ch * NT: ch * NT + NT], dst)

            n_used_cols = len({c for _, _, c in pair_jobs}
                              | {int(cols[s]) for s in single_jobs})
            NXB = n_used_cols + 1   # all cols stay resident within a rep
            NWS = max(Js, 1) + 2    # singles' weights prefetched early
            for _rep in range(reps):
                occ_idx = 0
                xtiles = {}

                def get_x(c, split_first=False):
                    if c in xtiles:
                        return xtiles[c]
                    xt = xpool.tile([128, XW], _DT, tag="xrow", bufs=NXB)
                    if split_first:
                        # split so the first matmuls start sooner
                        half = XW // 2
                        nc.sync.dma_start(xt[:, :half], xd[:, c, :half])
                        nc.sync.dma_start(xt[:, half:], xd[:, c, half:])
                    else:
                        nc.sync.dma_start(xt[:], xd[:, c])
                    xtiles[c] = xt
                    return xt

                # singles' weights are DMA'd one job ahead of use so the
                # interleaved single never waits on the ACT HWDGE queue
                swts = {}
                pi = 0   # pair ordinal (wp index)
                si = 0   # single ordinal (ws index)
                for ei, job in enumerate(emission):
                    if ei + 1 < len(emission) and emission[ei + 1][0] == "single":
                        nsi = sum(1 for jb in emission[:ei + 1]
                                  if jb[0] == "single")
                        swt = wpool.tile([128, 4, 128], _DT, tag="wts2",
                                         bufs=NWS)
                        nc.scalar.dma_start(swt[:], wds[:, nsi])
                        swts[nsi] = swt
                    if job[0] == "pair":
                        _, nL, nH, c = job
                        xrow = get_x(c, split_first=(pi == 0))
                        wt = wpool.tile([128, 8, 128], _DT, tag="wt")
                        if pi == 0:
                            # weights go through the idle ACT engine's HWDGE
                            # queue, in tap-pair chunks, so the first matmul
                            # issues as soon as its lhsT and x halves land
                            for jh in range(4):
                                nc.scalar.dma_start(
                                    wt[:, 2 * jh: 2 * jh + 2],
                                    wdp[:, pi, 2 * jh: 2 * jh + 2])
                        else:
                            nc.scalar.dma_start(wt[:], wdp[:, pi])
                        iL, iH = occ_idx, occ_idx + 1
                        occ_idx += 2
                        for ch in range(NCH):
                            ps = ppool.tile([128, NT], mybir.dt.float32,
                                            tag="ps", bufs=7)
                            for j in range(8):
                                nc.tensor.matmul(
                                    ps[:],
                                    _mm(wt[:, j, :]),
                                    _mm(xrow[:, ch * NT + 2 * j:
                                             ch * NT + 2 * j + NT]),
                                    start=(j == 0),
                                    stop=(j == 7),
                                )
                            drain(ps, iL, int(rows[nL]), 0, ch)
                            drain(ps, iH, int(rows[nH]), 1, ch)
                        pi += 1
                        continue
                    # single: tap-split self-pair, M=128 full array.
                    # M cols 0:64 = taps 0-7 (output-aligned, side 0); cols
                    # 64:128 = taps 8-15, landing at output t-8 (side 1, -8
                    # column shift in the drain; final 8 cols via tail job).
                    s = job[1]
                    c = int(cols[s])
                    r = int(rows[s])
                    x0 = get_x(c)
                    if si in swts:
                        wt = swts.pop(si)
                    else:
                        wt = wpool.tile([128, 4, 128], _DT, tag="wts2",
                                        bufs=NWS)
                        nc.scalar.dma_start(wt[:], wds[:, si])
                    iL, iH = occ_idx, occ_idx + 1
                    occ_idx += 2
                    sl0, sl1 = slots[0][r], slots[1][r]
                    if no_drain:
                        first0 = last0 = first1 = last1 = False
                        emit0 = emit1 = False
                    else:
                        first0, last0 = first[(r, 0)] == iL, last[(r, 0)] == iL
                        first1, last1 = first[(r, 1)] == iH, last[(r, 1)] == iH
                        emit0 = emit1 = True
                    for ch in range(NCH):
                        ps = ppool.tile([128, NT], mybir.dt.float32,
                                        tag="ps", bufs=7)
                        for j in range(4):
                            nc.tensor.matmul(
                                ps[:],
                                _mm(wt[:, j, :]),
                                _mm(x0[:, ch * NT + 2 * j:
                                       ch * NT + 2 * j + NT]),
                                start=(j == 0),
                                stop=(j == 3),
                            )
                        if emit0:
                            dstL = ya[0:64, sl0 * TH + ch * NT:
                                      sl0 * TH + ch * NT + NT]
                            if first0:
                                nc.scalar.activation(
                                    dstL, ps[0:64, :],
                                    mybir.ActivationFunctionType.Copy)
                            else:
                                nc.vector.tensor_add(out=dstL, in0=dstL,
                                                     in1=ps[0:64, :])
                            if last0:
                                nc.sync.dma_start(
                                    yd[0, sl0, :, ch * NT: ch * NT + NT],
                                    dstL)
                        if emit1:
                            # -8 column shift; psum cols mapping to t < t0
                            # belong to the previous core's range: dropped
                            if ch == 0:
                                srcH = ps[64:128, 8:NT]
                                lo, hi = 0, NT - 8
                            else:
                                srcH = ps[64:128, 0:NT]
                                lo, hi = ch * NT - 8, ch * NT + NT - 8
                            dstH = ya[64:128, sl1 * TH + lo: sl1 * TH + hi]
                            if first1:
                                nc.scalar.activation(
                                    dstH, srcH,
                                    mybir.ActivationFunctionType.Copy)
                            else:
                                nc.vector.tensor_add(out=dstH, in0=dstH,
                                                     in1=srcH)
                            if last1:
                                nc.sync.dma_start(yd[1, sl1, :, lo:hi],
                                                  dstH)
                    # tail: taps 8-15 of the final 8 output columns
                    pst = ppool.tile([128, 16], mybir.dt.float32,
                                     tag="pst", bufs=1)
                    for j in range(4):
                        nc.tensor.matmul(
                            pst[64:128, 0:8],
                            _mm(wt[:, j, 64:128]),
                            _mm(x0[:, NCH * NT + 2 * j:
                                   NCH * NT + 2 * j + 8]),
                            start=(j == 0),
                            stop=(j == 3),
                            tile_position=(0, 64),
                            skip_group_check=True,
                        )
                    if emit1:
                        dstT = ya[64:128, sl1 * TH + TH - 8: sl1 * TH + TH]
                        if first1:
                            nc.scalar.activation(
                                dstT, pst[64:128, 0:8],
                                mybir.ActivationFunctionType.Copy)
                        else:
                            nc.vector.tensor_add(out=dstT, in0=dstT,
                                                 in1=pst[64:128, 0:8])
                        if last1:
                            nc.sync.dma_start(yd[1, sl1, :, TH - 8: TH],
                                              dstT)
                    si += 1
    nc.compile()
    return nc, slots


_PROGRAM_CACHE = {}


def kernel(x, block_values, cols, rows):
    global LAST_EXEC_TIME_NS
    x = np.asarray(x)
    block_values = np.asarray(block_values)
    cols = np.asarray(cols)
    rows = np.asarray(rows)
    assert x.shape == (B, C, T) and block_values.shape == (NB, BS, BS, KS)

    pair_jobs, single_jobs = _build_schedule(cols, rows)
    wp, ws = _prep_weights(block_values.astype(np.float32), pair_jobs,
                           single_jobs)
    cache_key = (cols.tobytes(), rows.tobytes())
    if cache_key in _PROGRAM_CACHE:
        nc, slots = _PROGRAM_CACHE[cache_key]
    else:
        nc, slots = _build_program(pair_jobs, single_jobs, cols, rows)
        _PROGRAM_CACHE[cache_key] = (nc, slots)

    in_maps = []
    for core in range(N_CORES):
        b, h = divmod(core, 2)
        in_maps.append({"xp": _prep_x_core(x, b, h), "wp": wp, "ws": ws})

    res = run_bass_kernel_spmd(nc, in_maps, core_ids=list(range(N_CORES)))
    LAST_EXEC_TIME_NS = res.exec_time_ns

    y = np.zeros((B, C, T), np.float32)
    for core in range(N_CORES):
        b, h = divmod(core, 2)
        yc = res.results[core]["y"]  # (2, SMAX, 64, TH)
        for sidev in (0, 1):
            for r, s in slots[sidev].items():
                y[b, r * BS:(r + 1) * BS, h * TH:(h + 1) * TH] += yc[sidev, s]
    return y.astype(x.dtype, copy=False)


if __name__ == "__main__":
    import jax
    import reference

    with jax.default_device(jax.devices("cpu")[0]):
        inputs = reference.setup_inputs()
        np_inputs = {k: np.asarray(v) for k, v in inputs.items()}
        expected = np.asarray(reference.reference(**inputs))
    got = kernel(**np_inputs)
    rel = np.linalg.norm(got - expected) / np.linalg.norm(expected)
    print(f"Relative error: {rel:.3e}")

